# revision 4
# baseline (speedup 1.0000x reference)
"""Trainium2 Bass kernel for nn_Iterative_Model (retrieval_knn).

Reference computation:
    feats = x @ W_feat                      [8192, 2048]
    out   = feats @ W_fc + b_fc             [8192, 5]
    yhat  = argmax_c mean_s cos(feats, prototypes[c,s])   (epoch > 2)
          = argmax(out)                                   (epoch <= 2)

Algebraic fold (exact reassociation; argmax is invariant to the positive
per-row 1/||feats|| scale, so feats normalization drops out):
    G = [W_fc | P]  with P[:, c] = sum_s normalize(prototypes[c, s])   [2048, 10]
    D = W_feat @ G                                                     [4096, 10]
    R = x @ D;  out = R[:, :5] + b_fc;  yhat = argmax(R[:, 5:10])

Device plan (8 NeuronCores, data-parallel over batch):
  Launch A: core c computes D rows [512c, 512c+512) from its W_feat^T shard
            (each core reads 1/8 of W_feat; no collective needed — the tiny
            160 KB D is assembled on host between launches).
  Launch B: core c streams its x^T shard [4096, 1024] and accumulates
            R^T = D^T x^T in PSUM over 32 K-chunks (fp32), adds bias,
            PE-transposes 128-row chunks and computes both argmaxes on DVE.

Numerics: fp32 throughout; validated against the fixed-seed reference —
min top-2 score gap is ~100x larger than worst-case fp32 fold error
(0 label mismatches).
"""

import numpy as np

F32 = None  # set lazily (concourse import is heavy; keep module import cheap)
_PROGS = {}

NCORES = 8
B, DIN, DMID = 8192, 4096, 2048
BLOC = B // NCORES          # 1024 rows of x per core
ASH = DIN // NCORES         # 512 rows of D per core
NB = 512                    # b-chunk (moving free dim; fp32 max is 512)
NKO = DIN // 128            # 32 k-chunks for the main matmul
NOUT = 10                   # 5 logits | 5 prototype scores


def _build_programs():
    import concourse.bacc as bacc
    import concourse.mybir as mybir
    import concourse.tile as tile
    from concourse.masks import make_identity

    f32 = mybir.dt.float32
    AX = mybir.AxisListType.X
    OP = mybir.AluOpType

    # ---------------- Launch A: D shard = (W_feat^T shard)^T @ G ----------------
    ncA = bacc.Bacc("TRN2", target_bir_lowering=False, debug=False,
                    enable_asserts=False, num_devices=NCORES)
    wfT = ncA.dram_tensor("wfT", [DMID, ASH], f32, kind="ExternalInput")
    gA = ncA.dram_tensor("g", [DMID, NOUT], f32, kind="ExternalInput")
    d_out = ncA.dram_tensor("d_out", [128, ASH // 128, NOUT], f32, kind="ExternalOutput")
    NKD = DMID // 128  # 16 contraction chunks
    with tile.TileContext(ncA) as tc:
        with (
            tc.tile_pool(name="sb", bufs=1) as sb,
            tc.tile_pool(name="ps", bufs=2, space="PSUM") as ps,
        ):
            wfT_sb = sb.tile([128, NKD, ASH], f32)
            g_sb = sb.tile([128, NKD, NOUT], f32)
            ncA.sync.dma_start(wfT_sb[:], wfT.rearrange("(k p) a -> p k a", p=128))
            ncA.sync.dma_start(g_sb[:], gA.rearrange("(k p) j -> p k j", p=128))
            dsb = sb.tile([128, ASH // 128, NOUT], f32)
            for t in range(ASH // 128):
                psum_d = ps.tile([128, NOUT], f32)
                for kd in range(NKD):
                    ncA.tensor.matmul(
                        psum_d[:],
                        wfT_sb[:, kd, 128 * t:128 * (t + 1)],
                        g_sb[:, kd, :],
                        start=(kd == 0), stop=(kd == NKD - 1),
                    )
                ncA.vector.tensor_copy(dsb[:, t, :], psum_d[:])
            ncA.sync.dma_start(d_out[:], dsb[:])
    ncA.compile()

    # ---------------- Launch B: R^T = D^T @ x^T, bias, argmaxes ----------------
    ncB = bacc.Bacc("TRN2", target_bir_lowering=False, debug=False,
                    enable_asserts=False, num_devices=NCORES)
    xT = ncB.dram_tensor("xT", [DIN, BLOC], f32, kind="ExternalInput")
    d_in = ncB.dram_tensor("d", [128, NKO, NOUT], f32, kind="ExternalInput")
    b10 = ncB.dram_tensor("b10", [NOUT, 1], f32, kind="ExternalInput")
    iota = ncB.dram_tensor("iota", [1, 8], f32, kind="ExternalInput")
    out_t = ncB.dram_tensor("out_t", [NOUT, BLOC], f32, kind="ExternalOutput")
    yidx = ncB.dram_tensor("yidx", [128, BLOC // 128, 2], f32, kind="ExternalOutput")
    NT = BLOC // 128  # 8 transpose/argmax chunks
    with tile.TileContext(ncB) as tc:
        with (
            tc.tile_pool(name="const", bufs=1) as const,
            tc.tile_pool(name="xp", bufs=12) as xp,
            tc.tile_pool(name="vp", bufs=4) as vp,
            tc.tile_pool(name="pr", bufs=2, space="PSUM") as pr,
            tc.tile_pool(name="pt", bufs=2, space="PSUM") as pt,
        ):
            d_sb = const.tile([128, NKO, NOUT], f32)
            ncB.sync.dma_start(d_sb[:], d_in[:])
            b_sb = const.tile([NOUT, 1], f32)
            ncB.sync.dma_start(b_sb[:], b10[:])
            iota_sb = const.tile([128, 8], f32)
            ncB.sync.dma_start(iota_sb[:], iota[:].to_broadcast((128, 8)))
            ident = const.tile([NOUT, NOUT], f32)
            make_identity(ncB, ident[:])
            rt_sb = const.tile([NOUT, BLOC], f32)
            ym_sb = const.tile([128, NT, 2], f32)

            for bc in range(BLOC // NB):
                psum_r = pr.tile([NOUT, NB], f32)
                for ko in range(NKO):
                    xt = xp.tile([128, NB], f32)
                    ncB.sync.dma_start(
                        xt[:], xT[128 * ko:128 * (ko + 1), NB * bc:NB * (bc + 1)])
                    ncB.tensor.matmul(
                        psum_r[:], d_sb[:, ko, :], xt[:],
                        start=(ko == 0), stop=(ko == NKO - 1),
                    )
                # rt = psum + bias (bias rows 5..9 are zero)
                ncB.vector.tensor_tensor(
                    rt_sb[:, NB * bc:NB * (bc + 1)], psum_r[:],
                    b_sb[:].to_broadcast((NOUT, NB)), OP.add)
            ncB.sync.dma_start(out_t[:], rt_sb[:])

            for t in range(NT):
                tr = pt.tile([128, NOUT], f32)
                ncB.tensor.transpose(tr[:], rt_sb[:, 128 * t:128 * (t + 1)], ident[:])
                for grp, c0 in ((0, 5), (1, 0)):
                    mx = vp.tile([128, 1], f32, tag="mx")
                    ncB.vector.tensor_reduce(mx[:], tr[:, c0:c0 + 5], AX, OP.max)
                    eq = vp.tile([128, 5], f32, tag="eq")
                    ncB.vector.tensor_tensor(
                        eq[:], tr[:, c0:c0 + 5], mx[:].to_broadcast((128, 5)),
                        OP.is_equal)
                    # cand_j = (iota_j + 99) - 99*eq_j: equals j where tr==max,
                    # else >= 99 -> reduce_min picks the FIRST argmax (ties incl.)
                    ncB.vector.tensor_scalar_mul(eq[:], eq[:], -99.0)
                    ncB.vector.tensor_tensor(eq[:], eq[:], iota_sb[:, :5], OP.add)
                    ncB.vector.tensor_reduce(
                        ym_sb[:, t, grp:grp + 1], eq[:], AX, OP.min)
            ncB.sync.dma_start(yidx[:], ym_sb[:])
    ncB.compile()

    return {"A": ncA, "B": ncB}


def _get_progs():
    global _PROGS
    if not _PROGS:
        _PROGS = _build_programs()
    return _PROGS


def kernel(x, W_feat, W_fc, b_fc, prototypes, epoch):
    from concourse.bass_utils import run_bass_kernel_spmd

    progs = _get_progs()
    x = np.ascontiguousarray(np.asarray(x, np.float32))
    W_feat = np.asarray(W_feat, np.float32)
    W_fc = np.asarray(W_fc, np.float32)
    b_fc = np.asarray(b_fc, np.float32)
    prototypes = np.asarray(prototypes, np.float32)
    ep = int(np.asarray(epoch))

    # host prep (tiny): folded G = [W_fc | sum_s normalize(prototypes)]
    pr64 = prototypes.astype(np.float64)
    pn = pr64 / np.linalg.norm(pr64, axis=-1, keepdims=True)
    G = np.concatenate([W_fc.astype(np.float64), pn.sum(axis=1).T], axis=1)
    G = np.ascontiguousarray(G, np.float32)                      # [2048, 10]
    b10 = np.concatenate([b_fc, np.zeros(5, np.float32)]).reshape(NOUT, 1)
    iota = (np.arange(8, dtype=np.float32) + 99.0).reshape(1, 8)

    core_ids = list(range(NCORES))

    # ---- launch A: D shards ----
    in_A = [{
        "wfT": np.ascontiguousarray(W_feat[ASH * c:ASH * (c + 1), :].T),
        "g": G,
    } for c in core_ids]
    resA = run_bass_kernel_spmd(progs["A"], in_A, core_ids=core_ids, trace=False)
    # D in lhsT layout: d[p, ko, j] = D[128*ko + p, j]
    D = np.empty((128, NKO, NOUT), np.float32)
    for c in core_ids:
        D[:, 4 * c:4 * (c + 1), :] = resA.results[c]["d_out"]

    # ---- launch B: main matmul + argmaxes ----
    in_B = [{
        "xT": np.ascontiguousarray(x[BLOC * c:BLOC * (c + 1), :].T),
        "d": D, "b10": b10, "iota": iota,
    } for c in core_ids]
    resB = run_bass_kernel_spmd(progs["B"], in_B, core_ids=core_ids, trace=False)

    out = np.empty((B, 5), np.float32)
    yhat = np.empty(B, np.int32)
    grp = 0 if ep > 2 else 1
    for c in core_ids:
        r = resB.results[c]
        out[BLOC * c:BLOC * (c + 1), :] = r["out_t"][:5, :].T
        yhat[BLOC * c:BLOC * (c + 1)] = \
            r["yidx"][:, :, grp].T.reshape(BLOC).astype(np.int32)
    return out, yhat


# revision 11
# speedup vs baseline: 133.6169x; 133.6169x over previous
"""Trainium2 Bass kernel for nn_Iterative_Model (retrieval_knn).

Reference computation:
    feats = x @ W_feat                      [8192, 2048]
    out   = feats @ W_fc + b_fc             [8192, 5]
    yhat  = argmax_c mean_s cos(feats, prototypes[c,s])   (epoch > 2)
          = argmax(out)                                   (epoch <= 2)

Algebraic fold (exact reassociation; argmax is invariant to the positive
per-row 1/||feats|| scale, so feats normalization drops out):
    G = [W_fc | P]  with P[:, c] = sum_s normalize(prototypes[c, s])   [2048, 10]
    D = W_feat @ G                                                     [4096, 10]
    R = x @ D;  out = R[:, :5] + b_fc;  yhat = argmax(R[:, 5:10])

Device plan (8 NeuronCores, data-parallel over batch):
  Launch A: core c computes D rows [512c, 512c+512) from its W_feat^T shard
            (each core reads 1/8 of W_feat; no collective needed — the tiny
            160 KB D is assembled on host between launches).
  Launch B: core c streams its x^T shard [4096, 1024] and accumulates
            R^T = D^T x^T in PSUM over 32 K-chunks (fp32), adds bias,
            PE-transposes 128-row chunks and computes both argmaxes on DVE.

Numerics: fp32 throughout; validated against the fixed-seed reference —
min top-2 score gap is ~100x larger than worst-case fp32 fold error
(0 label mismatches).
"""

import numpy as np

F32 = None  # set lazily (concourse import is heavy; keep module import cheap)
_PROGS = {}

NCORES = 8
B, DIN, DMID = 8192, 4096, 2048
BLOC = B // NCORES          # 1024 rows of x per core
ASH = DIN // NCORES         # 512 rows of D per core
NB = 512                    # b-chunk (moving free dim; fp32 max is 512)
NKO = DIN // 128            # 32 k-chunks for the main matmul
NOUT = 10                   # 5 logits | 5 prototype scores


def _build_A():
    import concourse.bacc as bacc
    import concourse.mybir as mybir
    import concourse.tile as tile

    f32 = mybir.dt.float32

    # ---------------- Launch A: D shard = (W_feat^T shard)^T @ G ----------------
    ncA = bacc.Bacc("TRN2", target_bir_lowering=False, debug=False,
                    enable_asserts=False, num_devices=NCORES)
    wfT = ncA.dram_tensor("wfT", [DMID, ASH], f32, kind="ExternalInput")
    gA = ncA.dram_tensor("g", [DMID, NOUT], f32, kind="ExternalInput")
    d_out = ncA.dram_tensor("d_out", [128, ASH // 128, NOUT], f32, kind="ExternalOutput")
    NKD = DMID // 128  # 16 contraction chunks
    with tile.TileContext(ncA) as tc:
        with (
            tc.tile_pool(name="sb", bufs=1) as sb,
            tc.tile_pool(name="ps", bufs=2, space="PSUM") as ps,
        ):
            wfT_sb = sb.tile([128, NKD, ASH], f32)
            g_sb = sb.tile([128, NKD, NOUT], f32)
            ncA.sync.dma_start(wfT_sb[:], wfT.rearrange("(k p) a -> p k a", p=128))
            ncA.sync.dma_start(g_sb[:], gA.rearrange("(k p) j -> p k j", p=128))
            dsb = sb.tile([128, ASH // 128, NOUT], f32)
            for t in range(ASH // 128):
                psum_d = ps.tile([128, NOUT], f32)
                for kd in range(NKD):
                    ncA.tensor.matmul(
                        psum_d[:],
                        wfT_sb[:, kd, 128 * t:128 * (t + 1)],
                        g_sb[:, kd, :],
                        start=(kd == 0), stop=(kd == NKD - 1),
                    )
                ncA.vector.tensor_copy(dsb[:, t, :], psum_d[:])
            ncA.sync.dma_start(d_out[:], dsb[:])
    ncA.compile()
    return ncA


def _build_B(kg=4, rep=1, xbufs=6):
    """Launch B: R^T = D^T @ x^T, bias, argmaxes.

    kg: ko-chunks per x DMA (kg=4 -> 8 DMAs of 2 MB each; one HWDGE
        InstDMACopy splits across all 16 SDMA engine slots).
    rep: repetitions of the whole compute (timing harness only; rep=1 for real).
    """
    import concourse.bacc as bacc
    import concourse.mybir as mybir
    import concourse.tile as tile
    from concourse.masks import make_identity

    f32 = mybir.dt.float32
    AX = mybir.AxisListType.X
    OP = mybir.AluOpType

    ncB = bacc.Bacc("TRN2", target_bir_lowering=False, debug=False,
                    enable_asserts=False, num_devices=NCORES)
    xT = ncB.dram_tensor("xT", [DIN, BLOC], f32, kind="ExternalInput")
    d_in = ncB.dram_tensor("d", [128, NKO, NOUT], f32, kind="ExternalInput")
    b10 = ncB.dram_tensor("b10", [NOUT, 1], f32, kind="ExternalInput")
    iota = ncB.dram_tensor("iota", [1, 8], f32, kind="ExternalInput")
    out_t = ncB.dram_tensor("out_t", [NOUT, BLOC], f32, kind="ExternalOutput")
    yidx = ncB.dram_tensor("yidx", [128, BLOC // 128, 2], f32, kind="ExternalOutput")
    NT = BLOC // 128  # 8 transpose/argmax chunks
    # b-major streaming: finish batch-half bc=0 (all 32 K-chunks) before bc=1,
    # so its bias-add + transposes + argmaxes overlap bc=1's DMA stream.
    # One DMA spans kg K-chunks x NB batch cols (kg=4, NB=512 -> 1 MiB, the
    # documented max-bandwidth threshold for a single dma_start).
    xTv = xT.rearrange("(g s p) b -> g p s b", p=128, s=kg)  # [NKO/kg, 128, kg, BLOC]
    NBC = BLOC // NB
    with tile.TileContext(ncB) as tc:
        with (
            tc.tile_pool(name="const", bufs=1) as const,
            tc.tile_pool(name="xp", bufs=xbufs) as xp,
            tc.tile_pool(name="rp", bufs=2) as rp,
            tc.tile_pool(name="vp", bufs=4) as vp,
            tc.tile_pool(name="pr", bufs=2, space="PSUM") as pr,
            tc.tile_pool(name="pt", bufs=2, space="PSUM") as pt,
        ):
            d_sb = const.tile([128, NKO, NOUT], f32)
            ncB.sync.dma_start(d_sb[:], d_in[:])
            b_sb = const.tile([NOUT, 1], f32)
            ncB.sync.dma_start(b_sb[:], b10[:])
            iota_sb = const.tile([128, 8], f32)
            ncB.sync.dma_start(iota_sb[:], iota[:].to_broadcast((128, 8)))
            ident = const.tile([NOUT, NOUT], f32)
            make_identity(ncB, ident[:])

            for r in range(rep):
                ym_sb = rp.tile([128, NT, 2], f32, tag="ym")
                for bc in range(NBC):
                    psum_r = pr.tile([NOUT, NB], f32, tag="psum_r")
                    for g in range(NKO // kg):
                        xt = xp.tile([128, kg, NB], f32, tag="xt")
                        ncB.sync.dma_start(
                            xt[:], xTv[g][:, :, NB * bc:NB * (bc + 1)])
                        for s in range(kg):
                            ko = g * kg + s
                            ncB.tensor.matmul(
                                psum_r[:], d_sb[:, ko, :], xt[:, s, :],
                                start=(ko == 0), stop=(ko == NKO - 1),
                            )
                    # rt = psum + bias (bias rows 5..9 are zero)
                    rt_sb = rp.tile([NOUT, NB], f32, tag="rt")
                    ncB.vector.tensor_tensor(
                        rt_sb[:], psum_r[:],
                        b_sb[:].to_broadcast((NOUT, NB)), OP.add)
                    ncB.sync.dma_start(out_t[:, NB * bc:NB * (bc + 1)], rt_sb[:])

                    for tl in range(NB // 128):
                        t = bc * (NB // 128) + tl
                        tr = pt.tile([128, NOUT], f32)
                        ncB.tensor.transpose(
                            tr[:], rt_sb[:, 128 * tl:128 * (tl + 1)], ident[:])
                        for grp, c0 in ((0, 5), (1, 0)):
                            mx = vp.tile([128, 1], f32, tag="mx")
                            ncB.vector.tensor_reduce(
                                mx[:], tr[:, c0:c0 + 5], AX, OP.max)
                            eq = vp.tile([128, 5], f32, tag="eq")
                            ncB.vector.tensor_tensor(
                                eq[:], tr[:, c0:c0 + 5],
                                mx[:].to_broadcast((128, 5)), OP.is_equal)
                            # cand_j = (iota_j+99) - 99*eq_j: j where tr==max
                            # else >=99 -> reduce_min = FIRST argmax (jnp ties)
                            ncB.vector.tensor_scalar_mul(eq[:], eq[:], -99.0)
                            ncB.vector.tensor_tensor(
                                eq[:], eq[:], iota_sb[:, :5], OP.add)
                            ncB.vector.tensor_reduce(
                                ym_sb[:, t, grp:grp + 1], eq[:], AX, OP.min)
                ncB.sync.dma_start(yidx[:], ym_sb[:])
    ncB.compile()
    return ncB


def _get_progs():
    global _PROGS
    if not _PROGS:
        _PROGS = {"A": _build_A(), "B": _build_B()}
    return _PROGS


def kernel(x, W_feat, W_fc, b_fc, prototypes, epoch):
    from concourse.bass_utils import run_bass_kernel_spmd

    progs = _get_progs()
    x = np.ascontiguousarray(np.asarray(x, np.float32))
    W_feat = np.asarray(W_feat, np.float32)
    W_fc = np.asarray(W_fc, np.float32)
    b_fc = np.asarray(b_fc, np.float32)
    prototypes = np.asarray(prototypes, np.float32)
    ep = int(np.asarray(epoch))

    # host prep (tiny): folded G = [W_fc | sum_s normalize(prototypes)]
    pr64 = prototypes.astype(np.float64)
    pn = pr64 / np.linalg.norm(pr64, axis=-1, keepdims=True)
    G = np.concatenate([W_fc.astype(np.float64), pn.sum(axis=1).T], axis=1)
    G = np.ascontiguousarray(G, np.float32)                      # [2048, 10]
    b10 = np.concatenate([b_fc, np.zeros(5, np.float32)]).reshape(NOUT, 1)
    iota = (np.arange(8, dtype=np.float32) + 99.0).reshape(1, 8)

    core_ids = list(range(NCORES))

    # ---- launch A: D shards ----
    in_A = [{
        "wfT": np.ascontiguousarray(W_feat[ASH * c:ASH * (c + 1), :].T),
        "g": G,
    } for c in core_ids]
    resA = run_bass_kernel_spmd(progs["A"], in_A, core_ids=core_ids, trace=False)
    # D in lhsT layout: d[p, ko, j] = D[128*ko + p, j]
    D = np.empty((128, NKO, NOUT), np.float32)
    for c in core_ids:
        D[:, 4 * c:4 * (c + 1), :] = resA.results[c]["d_out"]

    # ---- launch B: main matmul + argmaxes ----
    in_B = [{
        "xT": np.ascontiguousarray(x[BLOC * c:BLOC * (c + 1), :].T),
        "d": D, "b10": b10, "iota": iota,
    } for c in core_ids]
    resB = run_bass_kernel_spmd(progs["B"], in_B, core_ids=core_ids, trace=False)

    out = np.empty((B, 5), np.float32)
    yhat = np.empty(B, np.int32)
    grp = 0 if ep > 2 else 1
    for c in core_ids:
        r = resB.results[c]
        out[BLOC * c:BLOC * (c + 1), :] = r["out_t"][:5, :].T
        yhat[BLOC * c:BLOC * (c + 1)] = \
            r["yidx"][:, :, grp].T.reshape(BLOC).astype(np.int32)
    return out, yhat


# revision 12
# speedup vs baseline: 136.2546x; 1.0197x over previous
"""Trainium2 Bass kernel for nn_Iterative_Model (retrieval_knn).

Reference computation:
    feats = x @ W_feat                      [8192, 2048]
    out   = feats @ W_fc + b_fc             [8192, 5]
    yhat  = argmax_c mean_s cos(feats, prototypes[c,s])   (epoch > 2)
          = argmax(out)                                   (epoch <= 2)

Algebraic fold (exact reassociation; argmax is invariant to the positive
per-row 1/||feats|| scale, so feats normalization drops out):
    G = [W_fc | P]  with P[:, c] = sum_s normalize(prototypes[c, s])   [2048, 10]
    D = W_feat @ G                                                     [4096, 10]
    R = x @ D;  out = R[:, :5] + b_fc;  yhat = argmax(R[:, 5:10])

Device plan (8 NeuronCores, data-parallel over batch):
  Launch A: core c computes D rows [512c, 512c+512) from its W_feat^T shard
            (each core reads 1/8 of W_feat; no collective needed — the tiny
            160 KB D is assembled on host between launches).
  Launch B: core c streams its x^T shard [4096, 1024] and accumulates
            R^T = D^T x^T in PSUM over 32 K-chunks (fp32), adds bias,
            PE-transposes 128-row chunks and computes both argmaxes on DVE.

Numerics: fp32 throughout; validated against the fixed-seed reference —
min top-2 score gap is ~100x larger than worst-case fp32 fold error
(0 label mismatches).
"""

import numpy as np

F32 = None  # set lazily (concourse import is heavy; keep module import cheap)
_PROGS = {}

NCORES = 8
B, DIN, DMID = 8192, 4096, 2048
BLOC = B // NCORES          # 1024 rows of x per core
ASH = DIN // NCORES         # 512 rows of D per core
NB = 512                    # b-chunk (moving free dim; fp32 max is 512)
NKO = DIN // 128            # 32 k-chunks for the main matmul
NOUT = 10                   # 5 logits | 5 prototype scores


def _build_A():
    import concourse.bacc as bacc
    import concourse.mybir as mybir
    import concourse.tile as tile

    f32 = mybir.dt.float32

    # ---------------- Launch A: D shard = (W_feat^T shard)^T @ G ----------------
    ncA = bacc.Bacc("TRN2", target_bir_lowering=False, debug=False,
                    enable_asserts=False, num_devices=NCORES)
    wfT = ncA.dram_tensor("wfT", [DMID, ASH], f32, kind="ExternalInput")
    gA = ncA.dram_tensor("g", [DMID, NOUT], f32, kind="ExternalInput")
    d_out = ncA.dram_tensor("d_out", [128, ASH // 128, NOUT], f32, kind="ExternalOutput")
    NKD = DMID // 128  # 16 contraction chunks
    with tile.TileContext(ncA) as tc:
        with (
            tc.tile_pool(name="sb", bufs=1) as sb,
            tc.tile_pool(name="ps", bufs=2, space="PSUM") as ps,
        ):
            g_sb = sb.tile([128, NKD, NOUT], f32)
            ncA.sync.dma_start(g_sb[:], gA.rearrange("(k p) j -> p k j", p=128))
            dsb = sb.tile([128, ASH // 128, NOUT], f32)
            wfTv = wfT.rearrange("(k p) a -> p k a", p=128)
            with tc.tile_pool(name="wp", bufs=2) as wp:
                # split the 4 MB W_feat-shard load per a-chunk (1 MB each) so
                # a-chunk t's matmuls overlap a-chunk t+1's DMA
                for t in range(ASH // 128):
                    wft = wp.tile([128, NKD, 128], f32, tag="wft")
                    ncA.sync.dma_start(wft[:], wfTv[:, :, 128 * t:128 * (t + 1)])
                    psum_d = ps.tile([128, NOUT], f32)
                    for kd in range(NKD):
                        ncA.tensor.matmul(
                            psum_d[:], wft[:, kd, :], g_sb[:, kd, :],
                            start=(kd == 0), stop=(kd == NKD - 1),
                        )
                    ncA.vector.tensor_copy(dsb[:, t, :], psum_d[:])
            ncA.sync.dma_start(d_out[:], dsb[:])
    ncA.compile()
    return ncA


def _build_B(kg=4, rep=1, xbufs=6):
    """Launch B: R^T = D^T @ x^T, bias, argmaxes.

    kg: ko-chunks per x DMA (kg=4 -> 8 DMAs of 2 MB each; one HWDGE
        InstDMACopy splits across all 16 SDMA engine slots).
    rep: repetitions of the whole compute (timing harness only; rep=1 for real).
    """
    import concourse.bacc as bacc
    import concourse.mybir as mybir
    import concourse.tile as tile
    from concourse.masks import make_identity

    f32 = mybir.dt.float32
    AX = mybir.AxisListType.X
    OP = mybir.AluOpType

    ncB = bacc.Bacc("TRN2", target_bir_lowering=False, debug=False,
                    enable_asserts=False, num_devices=NCORES)
    xT = ncB.dram_tensor("xT", [DIN, BLOC], f32, kind="ExternalInput")
    d_in = ncB.dram_tensor("d", [128, NKO, NOUT], f32, kind="ExternalInput")
    b10 = ncB.dram_tensor("b10", [NOUT, 1], f32, kind="ExternalInput")
    iota = ncB.dram_tensor("iota", [1, 8], f32, kind="ExternalInput")
    out_t = ncB.dram_tensor("out_t", [NOUT, BLOC], f32, kind="ExternalOutput")
    yidx = ncB.dram_tensor("yidx", [128, BLOC // 128, 2], f32, kind="ExternalOutput")
    NT = BLOC // 128  # 8 transpose/argmax chunks
    # b-major streaming: finish batch-half bc=0 (all 32 K-chunks) before bc=1,
    # so its bias-add + transposes + argmaxes overlap bc=1's DMA stream.
    # One DMA spans kg K-chunks x NB batch cols (kg=4, NB=512 -> 1 MiB, the
    # documented max-bandwidth threshold for a single dma_start).
    xTv = xT.rearrange("(g s p) b -> g p s b", p=128, s=kg)  # [NKO/kg, 128, kg, BLOC]
    NBC = BLOC // NB
    with tile.TileContext(ncB) as tc:
        with (
            tc.tile_pool(name="const", bufs=1) as const,
            tc.tile_pool(name="xp", bufs=xbufs) as xp,
            tc.tile_pool(name="rp", bufs=2) as rp,
            tc.tile_pool(name="vp", bufs=4) as vp,
            tc.tile_pool(name="pr", bufs=2, space="PSUM") as pr,
            tc.tile_pool(name="pt", bufs=2, space="PSUM") as pt,
        ):
            d_sb = const.tile([128, NKO, NOUT], f32)
            ncB.sync.dma_start(d_sb[:], d_in[:])
            b_sb = const.tile([NOUT, 1], f32)
            ncB.sync.dma_start(b_sb[:], b10[:])
            iota_sb = const.tile([128, 8], f32)
            ncB.sync.dma_start(iota_sb[:], iota[:].to_broadcast((128, 8)))
            ident = const.tile([NOUT, NOUT], f32)
            make_identity(ncB, ident[:])

            for r in range(rep):
                ym_sb = rp.tile([128, NT, 2], f32, tag="ym")
                for bc in range(NBC):
                    psum_r = pr.tile([NOUT, NB], f32, tag="psum_r")
                    for g in range(NKO // kg):
                        xt = xp.tile([128, kg, NB], f32, tag="xt")
                        ncB.sync.dma_start(
                            xt[:], xTv[g][:, :, NB * bc:NB * (bc + 1)])
                        for s in range(kg):
                            ko = g * kg + s
                            ncB.tensor.matmul(
                                psum_r[:], d_sb[:, ko, :], xt[:, s, :],
                                start=(ko == 0), stop=(ko == NKO - 1),
                            )
                    # rt = psum + bias (bias rows 5..9 are zero)
                    rt_sb = rp.tile([NOUT, NB], f32, tag="rt")
                    ncB.vector.tensor_tensor(
                        rt_sb[:], psum_r[:],
                        b_sb[:].to_broadcast((NOUT, NB)), OP.add)
                    ncB.sync.dma_start(out_t[:, NB * bc:NB * (bc + 1)], rt_sb[:])

                    for tl in range(NB // 128):
                        t = bc * (NB // 128) + tl
                        tr = pt.tile([128, NOUT], f32)
                        ncB.tensor.transpose(
                            tr[:], rt_sb[:, 128 * tl:128 * (tl + 1)], ident[:])
                        for grp, c0 in ((0, 5), (1, 0)):
                            mx = vp.tile([128, 1], f32, tag="mx")
                            ncB.vector.tensor_reduce(
                                mx[:], tr[:, c0:c0 + 5], AX, OP.max)
                            eq = vp.tile([128, 5], f32, tag="eq")
                            ncB.vector.tensor_tensor(
                                eq[:], tr[:, c0:c0 + 5],
                                mx[:].to_broadcast((128, 5)), OP.is_equal)
                            # cand_j = (iota_j+99) - 99*eq_j: j where tr==max
                            # else >=99 -> reduce_min = FIRST argmax (jnp ties)
                            ncB.vector.tensor_scalar_mul(eq[:], eq[:], -99.0)
                            ncB.vector.tensor_tensor(
                                eq[:], eq[:], iota_sb[:, :5], OP.add)
                            ncB.vector.tensor_reduce(
                                ym_sb[:, t, grp:grp + 1], eq[:], AX, OP.min)
                ncB.sync.dma_start(yidx[:], ym_sb[:])
    ncB.compile()
    return ncB


def _get_progs():
    global _PROGS
    if not _PROGS:
        _PROGS = {"A": _build_A(), "B": _build_B()}
    return _PROGS


def kernel(x, W_feat, W_fc, b_fc, prototypes, epoch):
    from concourse.bass_utils import run_bass_kernel_spmd

    progs = _get_progs()
    x = np.ascontiguousarray(np.asarray(x, np.float32))
    W_feat = np.asarray(W_feat, np.float32)
    W_fc = np.asarray(W_fc, np.float32)
    b_fc = np.asarray(b_fc, np.float32)
    prototypes = np.asarray(prototypes, np.float32)
    ep = int(np.asarray(epoch))

    # host prep (tiny): folded G = [W_fc | sum_s normalize(prototypes)]
    pr64 = prototypes.astype(np.float64)
    pn = pr64 / np.linalg.norm(pr64, axis=-1, keepdims=True)
    G = np.concatenate([W_fc.astype(np.float64), pn.sum(axis=1).T], axis=1)
    G = np.ascontiguousarray(G, np.float32)                      # [2048, 10]
    b10 = np.concatenate([b_fc, np.zeros(5, np.float32)]).reshape(NOUT, 1)
    iota = (np.arange(8, dtype=np.float32) + 99.0).reshape(1, 8)

    core_ids = list(range(NCORES))

    # ---- launch A: D shards ----
    in_A = [{
        "wfT": np.ascontiguousarray(W_feat[ASH * c:ASH * (c + 1), :].T),
        "g": G,
    } for c in core_ids]
    resA = run_bass_kernel_spmd(progs["A"], in_A, core_ids=core_ids, trace=False)
    # D in lhsT layout: d[p, ko, j] = D[128*ko + p, j]
    D = np.empty((128, NKO, NOUT), np.float32)
    for c in core_ids:
        D[:, 4 * c:4 * (c + 1), :] = resA.results[c]["d_out"]

    # ---- launch B: main matmul + argmaxes ----
    in_B = [{
        "xT": np.ascontiguousarray(x[BLOC * c:BLOC * (c + 1), :].T),
        "d": D, "b10": b10, "iota": iota,
    } for c in core_ids]
    resB = run_bass_kernel_spmd(progs["B"], in_B, core_ids=core_ids, trace=False)

    out = np.empty((B, 5), np.float32)
    yhat = np.empty(B, np.int32)
    grp = 0 if ep > 2 else 1
    for c in core_ids:
        r = resB.results[c]
        out[BLOC * c:BLOC * (c + 1), :] = r["out_t"][:5, :].T
        yhat[BLOC * c:BLOC * (c + 1)] = \
            r["yidx"][:, :, grp].T.reshape(BLOC).astype(np.int32)
    return out, yhat


# revision 14
# speedup vs baseline: 149.5746x; 1.0978x over previous
"""Trainium2 Bass kernel for nn_Iterative_Model (retrieval_knn).

Reference computation:
    feats = x @ W_feat                      [8192, 2048]
    out   = feats @ W_fc + b_fc             [8192, 5]
    yhat  = argmax_c mean_s cos(feats, prototypes[c,s])   (epoch > 2)
          = argmax(out)                                   (epoch <= 2)

Algebraic fold (exact reassociation; argmax is invariant to the positive
per-row 1/||feats|| scale, so feats normalization drops out):
    G = [W_fc | P]  with P[:, c] = sum_s normalize(prototypes[c, s])   [2048, 10]
    D = W_feat @ G                                                     [4096, 10]
    R = x @ D;  out = R[:, :5] + b_fc;  yhat = argmax(R[:, 5:10])

Device plan (8 NeuronCores, data-parallel over batch):
  Launch A: core c computes D rows [512c, 512c+512) from its W_feat^T shard
            (each core reads 1/8 of W_feat; no collective needed — the tiny
            160 KB D is assembled on host between launches).
  Launch B: core c streams its x^T shard [4096, 1024] and accumulates
            R^T = D^T x^T in PSUM over 32 K-chunks (fp32), adds bias,
            PE-transposes 128-row chunks and computes both argmaxes on DVE.

Numerics: fp32 throughout; validated against the fixed-seed reference —
min top-2 score gap is ~100x larger than worst-case fp32 fold error
(0 label mismatches).
"""

import numpy as np

F32 = None  # set lazily (concourse import is heavy; keep module import cheap)
_PROGS = {}

NCORES = 8
B, DIN, DMID = 8192, 4096, 2048
BLOC = B // NCORES          # 1024 rows of x per core
ASH = DIN // NCORES         # 512 rows of D per core
NB = 512                    # b-chunk (moving free dim; fp32 max is 512)
NKO = DIN // 128            # 32 k-chunks for the main matmul
NOUT = 10                   # 5 logits | 5 prototype scores


def _build_A():
    import concourse.bacc as bacc
    import concourse.mybir as mybir
    import concourse.tile as tile

    f32 = mybir.dt.float32

    # ---------------- Launch A: D shard = (W_feat^T shard)^T @ G ----------------
    ncA = bacc.Bacc("TRN2", target_bir_lowering=False, debug=False,
                    enable_asserts=False, num_devices=NCORES)
    wfT = ncA.dram_tensor("wfT", [DMID, ASH], f32, kind="ExternalInput")
    gA = ncA.dram_tensor("g", [DMID, NOUT], f32, kind="ExternalInput")
    d_out = ncA.dram_tensor("d_out", [128, ASH // 128, NOUT], f32, kind="ExternalOutput")
    NKD = DMID // 128  # 16 contraction chunks
    with tile.TileContext(ncA) as tc:
        with (
            tc.tile_pool(name="sb", bufs=1) as sb,
            tc.tile_pool(name="ps", bufs=2, space="PSUM") as ps,
        ):
            g_sb = sb.tile([128, NKD, NOUT], f32)
            ncA.sync.dma_start(g_sb[:], gA.rearrange("(k p) j -> p k j", p=128))
            dsb = sb.tile([128, ASH // 128, NOUT], f32)
            wfTv = wfT.rearrange("(k p) a -> p k a", p=128)
            with tc.tile_pool(name="wp", bufs=2) as wp:
                # split the 4 MB W_feat-shard load per a-chunk (1 MB each) so
                # a-chunk t's matmuls overlap a-chunk t+1's DMA
                for t in range(ASH // 128):
                    wft = wp.tile([128, NKD, 128], f32, tag="wft")
                    ncA.sync.dma_start(wft[:], wfTv[:, :, 128 * t:128 * (t + 1)])
                    psum_d = ps.tile([128, NOUT], f32)
                    for kd in range(NKD):
                        ncA.tensor.matmul(
                            psum_d[:], wft[:, kd, :], g_sb[:, kd, :],
                            start=(kd == 0), stop=(kd == NKD - 1),
                        )
                    ncA.vector.tensor_copy(dsb[:, t, :], psum_d[:])
            ncA.sync.dma_start(d_out[:], dsb[:])
    ncA.compile()
    return ncA


def _build_B(kg=4, rep=1, xbufs=6):
    """Launch B: R^T = D^T @ x^T, bias, argmaxes.

    kg: ko-chunks per x DMA (kg=4 -> 8 DMAs of 2 MB each; one HWDGE
        InstDMACopy splits across all 16 SDMA engine slots).
    rep: repetitions of the whole compute (timing harness only; rep=1 for real).
    """
    import concourse.bacc as bacc
    import concourse.mybir as mybir
    import concourse.tile as tile
    from concourse.masks import make_identity

    f32 = mybir.dt.float32
    AX = mybir.AxisListType.X
    OP = mybir.AluOpType

    ncB = bacc.Bacc("TRN2", target_bir_lowering=False, debug=False,
                    enable_asserts=False, num_devices=NCORES)
    xT = ncB.dram_tensor("xT", [DIN, BLOC], f32, kind="ExternalInput")
    d_in = ncB.dram_tensor("d", [128, NKO, NOUT], f32, kind="ExternalInput")
    b10 = ncB.dram_tensor("b10", [NOUT, 1], f32, kind="ExternalInput")
    iota = ncB.dram_tensor("iota", [1, 8], f32, kind="ExternalInput")
    out_t = ncB.dram_tensor("out_t", [NOUT, BLOC], f32, kind="ExternalOutput")
    yidx = ncB.dram_tensor("yidx", [128, BLOC // 128, 2], f32, kind="ExternalOutput")
    NT = BLOC // 128  # 8 transpose/argmax chunks
    # b-major streaming: finish batch-half bc=0 (all 32 K-chunks) before bc=1,
    # so its bias-add + transposes + argmaxes overlap bc=1's DMA stream.
    # One DMA spans kg K-chunks x NB batch cols (kg=4, NB=512 -> 1 MiB, the
    # documented max-bandwidth threshold for a single dma_start).
    xTv = xT.rearrange("(g s p) b -> g p s b", p=128, s=kg)  # [NKO/kg, 128, kg, BLOC]
    NBC = BLOC // NB
    with tile.TileContext(ncB) as tc:
        with (
            tc.tile_pool(name="const", bufs=1) as const,
            tc.tile_pool(name="xp", bufs=xbufs) as xp,
            tc.tile_pool(name="rp", bufs=2) as rp,
            tc.tile_pool(name="vp", bufs=4) as vp,
            tc.tile_pool(name="pr", bufs=2, space="PSUM") as pr,
            tc.tile_pool(name="pt", bufs=2, space="PSUM") as pt,
        ):
            d_sb = const.tile([128, NKO, NOUT], f32)
            ncB.sync.dma_start(d_sb[:], d_in[:])
            b_sb = const.tile([NOUT, 1], f32)
            ncB.sync.dma_start(b_sb[:], b10[:])
            iota_sb = const.tile([128, 8], f32)
            ncB.sync.dma_start(iota_sb[:], iota[:].to_broadcast((128, 8)))
            ident = const.tile([NOUT, NOUT], f32)
            make_identity(ncB, ident[:])

            for r in range(rep):
                ym_sb = rp.tile([128, NT, 2], f32, tag="ym")
                for bc in range(NBC):
                    psum_r = pr.tile([NOUT, NB], f32, tag="psum_r")
                    for g in range(NKO // kg):
                        xt = xp.tile([128, kg, NB], f32, tag="xt")
                        ncB.sync.dma_start(
                            xt[:], xTv[g][:, :, NB * bc:NB * (bc + 1)])
                        for s in range(kg):
                            ko = g * kg + s
                            ncB.tensor.matmul(
                                psum_r[:], d_sb[:, ko, :], xt[:, s, :],
                                start=(ko == 0), stop=(ko == NKO - 1),
                            )
                    # rt = psum + bias (bias rows 5..9 are zero)
                    rt_sb = rp.tile([NOUT, NB], f32, tag="rt")
                    ncB.vector.tensor_tensor(
                        rt_sb[:], psum_r[:],
                        b_sb[:].to_broadcast((NOUT, NB)), OP.add)
                    ncB.sync.dma_start(out_t[:, NB * bc:NB * (bc + 1)], rt_sb[:])

                    for tl in range(NB // 128):
                        t = bc * (NB // 128) + tl
                        tr = pt.tile([128, NOUT], f32)
                        ncB.tensor.transpose(
                            tr[:], rt_sb[:, 128 * tl:128 * (tl + 1)], ident[:])
                        for grp, c0 in ((0, 5), (1, 0)):
                            mx = vp.tile([128, 1], f32, tag="mx")
                            ncB.vector.tensor_reduce(
                                mx[:], tr[:, c0:c0 + 5], AX, OP.max)
                            eq = vp.tile([128, 5], f32, tag="eq")
                            ncB.vector.tensor_tensor(
                                eq[:], tr[:, c0:c0 + 5],
                                mx[:].to_broadcast((128, 5)), OP.is_equal)
                            # cand_j = (iota_j+99) - 99*eq_j: j where tr==max
                            # else >=99 -> reduce_min = FIRST argmax (jnp ties)
                            ncB.vector.tensor_scalar_mul(eq[:], eq[:], -99.0)
                            ncB.vector.tensor_tensor(
                                eq[:], eq[:], iota_sb[:, :5], OP.add)
                            ncB.vector.tensor_reduce(
                                ym_sb[:, t, grp:grp + 1], eq[:], AX, OP.min)
                ncB.sync.dma_start(yidx[:], ym_sb[:])
    ncB.compile()
    return ncB


def _build_B2(kg=8, rep=1, xbufs=6):
    """Launch B, split-precision: x = xh + xl, D = Dh + Dl (bf16 pairs).

    Four bf16 accumulation chains replace one fp32 chain: same DMA bytes
    (2 x 8.4 MB bf16 == 16.8 MB fp32) but 1 cyc/row on PE instead of 4
    (fp32 matmul = 2 half-speed passes). Residual error ~2^-17 << the
    ~7.5e-4 min top-2 argmax gap. kg=8 -> 1 MiB per bf16 dma_start.
    """
    import concourse.bacc as bacc
    import concourse.mybir as mybir
    import concourse.tile as tile
    from concourse.masks import make_identity

    f32 = mybir.dt.float32
    bf16 = mybir.dt.bfloat16
    AX = mybir.AxisListType.X
    OP = mybir.AluOpType

    ncB = bacc.Bacc("TRN2", target_bir_lowering=False, debug=False,
                    enable_asserts=False, num_devices=NCORES)
    xh = ncB.dram_tensor("xh", [DIN, BLOC], bf16, kind="ExternalInput")
    xl = ncB.dram_tensor("xl", [DIN, BLOC], bf16, kind="ExternalInput")
    dh = ncB.dram_tensor("dh", [128, NKO, NOUT], bf16, kind="ExternalInput")
    dl = ncB.dram_tensor("dl", [128, NKO, NOUT], bf16, kind="ExternalInput")
    b10 = ncB.dram_tensor("b10", [NOUT, 1], f32, kind="ExternalInput")
    iota = ncB.dram_tensor("iota", [1, 8], f32, kind="ExternalInput")
    out_t = ncB.dram_tensor("out_t", [NOUT, BLOC], f32, kind="ExternalOutput")
    yidx = ncB.dram_tensor("yidx", [128, BLOC // 128, 2], f32, kind="ExternalOutput")
    NT = BLOC // 128
    xhv = xh.rearrange("(g s p) b -> g p s b", p=128, s=kg)
    xlv = xl.rearrange("(g s p) b -> g p s b", p=128, s=kg)
    NBC = BLOC // NB
    with tile.TileContext(ncB) as tc:
        with (
            tc.tile_pool(name="const", bufs=1) as const,
            tc.tile_pool(name="xp", bufs=xbufs) as xp,
            tc.tile_pool(name="rp", bufs=2) as rp,
            tc.tile_pool(name="vp", bufs=4) as vp,
            tc.tile_pool(name="pr", bufs=2, space="PSUM") as pr,
            tc.tile_pool(name="pt", bufs=2, space="PSUM") as pt,
        ):
            dh_sb = const.tile([128, NKO, NOUT], bf16)
            ncB.sync.dma_start(dh_sb[:], dh[:])
            dl_sb = const.tile([128, NKO, NOUT], bf16)
            ncB.sync.dma_start(dl_sb[:], dl[:])
            b_sb = const.tile([NOUT, 1], f32)
            ncB.sync.dma_start(b_sb[:], b10[:])
            iota_sb = const.tile([128, 8], f32)
            ncB.sync.dma_start(iota_sb[:], iota[:].to_broadcast((128, 8)))
            ident = const.tile([NOUT, NOUT], f32)
            make_identity(ncB, ident[:])

            for r in range(rep):
                ym_sb = rp.tile([128, NT, 2], f32, tag="ym")
                for bc in range(NBC):
                    psum_r = pr.tile([NOUT, NB], f32, tag="psum_r")
                    for g in range(NKO // kg):
                        xht = xp.tile([128, kg, NB], bf16, tag="xht")
                        ncB.sync.dma_start(
                            xht[:], xhv[g][:, :, NB * bc:NB * (bc + 1)])
                        xlt = xp.tile([128, kg, NB], bf16, tag="xlt")
                        ncB.sync.dma_start(
                            xlt[:], xlv[g][:, :, NB * bc:NB * (bc + 1)])
                        for s in range(kg):
                            ko = g * kg + s
                            for i, (dsb, xt) in enumerate(
                                    ((dh_sb, xht), (dl_sb, xht),
                                     (dh_sb, xlt), (dl_sb, xlt))):
                                ncB.tensor.matmul(
                                    psum_r[:], dsb[:, ko, :], xt[:, s, :],
                                    start=(ko == 0 and i == 0),
                                    stop=(ko == NKO - 1 and i == 3),
                                )
                    rt_sb = rp.tile([NOUT, NB], f32, tag="rt")
                    ncB.vector.tensor_tensor(
                        rt_sb[:], psum_r[:],
                        b_sb[:].to_broadcast((NOUT, NB)), OP.add)
                    ncB.sync.dma_start(out_t[:, NB * bc:NB * (bc + 1)], rt_sb[:])

                    for tl in range(NB // 128):
                        t = bc * (NB // 128) + tl
                        tr = pt.tile([128, NOUT], f32)
                        ncB.tensor.transpose(
                            tr[:], rt_sb[:, 128 * tl:128 * (tl + 1)], ident[:])
                        for grp, c0 in ((0, 5), (1, 0)):
                            mx = vp.tile([128, 1], f32, tag="mx")
                            ncB.vector.tensor_reduce(
                                mx[:], tr[:, c0:c0 + 5], AX, OP.max)
                            eq = vp.tile([128, 5], f32, tag="eq")
                            ncB.vector.tensor_tensor(
                                eq[:], tr[:, c0:c0 + 5],
                                mx[:].to_broadcast((128, 5)), OP.is_equal)
                            ncB.vector.tensor_scalar_mul(eq[:], eq[:], -99.0)
                            ncB.vector.tensor_tensor(
                                eq[:], eq[:], iota_sb[:, :5], OP.add)
                            ncB.vector.tensor_reduce(
                                ym_sb[:, t, grp:grp + 1], eq[:], AX, OP.min)
                ncB.sync.dma_start(yidx[:], ym_sb[:])
    ncB.compile()
    return ncB


USE_SPLIT_BF16 = True


def _get_progs():
    global _PROGS
    if not _PROGS:
        _PROGS = {"A": _build_A(),
                  "B": _build_B2() if USE_SPLIT_BF16 else _build_B()}
    return _PROGS


def kernel(x, W_feat, W_fc, b_fc, prototypes, epoch):
    from concourse.bass_utils import run_bass_kernel_spmd

    progs = _get_progs()
    x = np.ascontiguousarray(np.asarray(x, np.float32))
    W_feat = np.asarray(W_feat, np.float32)
    W_fc = np.asarray(W_fc, np.float32)
    b_fc = np.asarray(b_fc, np.float32)
    prototypes = np.asarray(prototypes, np.float32)
    ep = int(np.asarray(epoch))

    # host prep (tiny): folded G = [W_fc | sum_s normalize(prototypes)]
    pr64 = prototypes.astype(np.float64)
    pn = pr64 / np.linalg.norm(pr64, axis=-1, keepdims=True)
    G = np.concatenate([W_fc.astype(np.float64), pn.sum(axis=1).T], axis=1)
    G = np.ascontiguousarray(G, np.float32)                      # [2048, 10]
    b10 = np.concatenate([b_fc, np.zeros(5, np.float32)]).reshape(NOUT, 1)
    iota = (np.arange(8, dtype=np.float32) + 99.0).reshape(1, 8)

    core_ids = list(range(NCORES))

    # ---- launch A: D shards ----
    in_A = [{
        "wfT": np.ascontiguousarray(W_feat[ASH * c:ASH * (c + 1), :].T),
        "g": G,
    } for c in core_ids]
    resA = run_bass_kernel_spmd(progs["A"], in_A, core_ids=core_ids, trace=False)
    # D in lhsT layout: d[p, ko, j] = D[128*ko + p, j]
    D = np.empty((128, NKO, NOUT), np.float32)
    for c in core_ids:
        D[:, 4 * c:4 * (c + 1), :] = resA.results[c]["d_out"]

    # ---- launch B: main matmul + argmaxes ----
    if USE_SPLIT_BF16:
        import ml_dtypes
        bf = ml_dtypes.bfloat16
        Dh = D.astype(bf)
        Dl = (D - Dh.astype(np.float32)).astype(bf)
        in_B = []
        for c in core_ids:
            xs = x[BLOC * c:BLOC * (c + 1), :]
            xsh = xs.astype(bf)
            xsl = (xs - xsh.astype(np.float32)).astype(bf)
            in_B.append({
                "xh": np.ascontiguousarray(xsh.T),
                "xl": np.ascontiguousarray(xsl.T),
                "dh": Dh, "dl": Dl, "b10": b10, "iota": iota,
            })
    else:
        in_B = [{
            "xT": np.ascontiguousarray(x[BLOC * c:BLOC * (c + 1), :].T),
            "d": D, "b10": b10, "iota": iota,
        } for c in core_ids]
    resB = run_bass_kernel_spmd(progs["B"], in_B, core_ids=core_ids, trace=False)

    out = np.empty((B, 5), np.float32)
    yhat = np.empty(B, np.int32)
    grp = 0 if ep > 2 else 1
    for c in core_ids:
        r = resB.results[c]
        out[BLOC * c:BLOC * (c + 1), :] = r["out_t"][:5, :].T
        yhat[BLOC * c:BLOC * (c + 1)] = \
            r["yidx"][:, :, grp].T.reshape(BLOC).astype(np.int32)
    return out, yhat
